# revision 10
# baseline (speedup 1.0000x reference)
"""Trainium2 Bass kernel for nn_AutoEncoderModular (MoE-routed grouped autoencoder).

Strategy: expert-parallel. Host sorts wafers by expert key into 16
capacity-padded buckets; core c handles experts (2c, 2c+1). Every stage is a
matmul with wafers in the moving/free dimension:

  - encoder conv (masked 3x3, groups=14, 1->8ch) is decomposed into 4 matmuls
    per (expert, group), one per 2x2 maxpool phase, with a host-built
    Toeplitz-x-weight stationary [64 in-pix, 128=(8ch x 16 pooled-pix)].
    ReLU commutes with max, so maxpool = elementwise max of the 4 phase tiles,
    which lands `feat` directly in the [128 features, wafers] layout the dense
    encoder needs (no transposes anywhere).
  - enc dense K=128->M=8, dec dense K=8->M=128, and the stride-2 grouped
    ConvTranspose as a host-built [K=128, M=64] linear map; bias+ReLU fused
    into ACT/DVE epilogue ops.
"""

import numpy as np

E, G, CPG = 16, 14, 8
N_CORES = 8
EPC = E // N_CORES  # experts per core

MM_DTYPE = "float16"  # "float32r" (TF32-like) or "float32" (exact)
_PROG_CACHE = {}
_LAST_IN_MAPS = None


# ---------------------------------------------------------------- tables
def _build_t_enc(conv_mask, enc_conv_w):
    """[E, G, 4, 64, 128] : phase (yb,xb); col m = co*16 + y2*4 + x2."""
    wm = (enc_conv_w[:, :, 0] * conv_mask).reshape(E, G, CPG, 3, 3)
    t = np.zeros((E, G, 4, 64, 128), np.float32)
    for ph in range(4):
        yb, xb = ph // 2, ph % 2
        for dy in range(3):
            for dx in range(3):
                if conv_mask[dy, dx] == 0.0:
                    continue
                for y2 in range(4):
                    yi = 2 * y2 + yb + dy - 1
                    if not (0 <= yi < 8):
                        continue
                    for x2 in range(4):
                        xi = 2 * x2 + xb + dx - 1
                        if not (0 <= xi < 8):
                            continue
                        pi = yi * 8 + xi
                        for co in range(CPG):
                            m = co * 16 + y2 * 4 + x2
                            t[:, :, ph, pi, m] += wm[:, :, co, dy, dx]
    return t


def _build_t_dec(dec_tconv_w):
    """[E, G, 128, 64] : ConvTranspose2d(s=2, p=1, op=1) as a linear map.
    in f = c*16 + yi*4 + xi ; out po = yo*8 + xo ; yo = 2*yi + ky - 1."""
    w = dec_tconv_w[:, :, 0].reshape(E, G, CPG, 3, 3)
    t = np.zeros((E, G, 128, 64), np.float32)
    for c in range(CPG):
        for yi in range(4):
            for xi in range(4):
                f = c * 16 + yi * 4 + xi
                for ky in range(3):
                    yo = 2 * yi + ky - 1
                    if not (0 <= yo < 8):
                        continue
                    for kx in range(3):
                        xo = 2 * xi + kx - 1
                        if not (0 <= xo < 8):
                            continue
                        t[:, :, f, yo * 8 + xo] += w[:, :, c, ky, kx]
    return t


# ---------------------------------------------------------------- device program
def _build_program(cap):
    import concourse.tile as tile
    from concourse import bacc, mybir

    f32 = mybir.dt.float32
    mmdt = getattr(mybir.dt, MM_DTYPE)
    nc = bacc.Bacc("TRN2", target_bir_lowering=False)

    xt = nc.dram_tensor("xt", [64, EPC, G, cap], mmdt, kind="ExternalInput")
    th = nc.dram_tensor("th", [64, EPC, G, 4, 128], mmdt, kind="ExternalInput")
    weh = nc.dram_tensor("weh", [128, EPC, G, 8], mmdt, kind="ExternalInput")
    beh = nc.dram_tensor("beh", [8, EPC, G], f32, kind="ExternalInput")
    wdh = nc.dram_tensor("wdh", [8, EPC, G, 128], mmdt, kind="ExternalInput")
    bdh = nc.dram_tensor("bdh", [128, EPC, G], f32, kind="ExternalInput")
    tdh = nc.dram_tensor("tdh", [128, EPC, G, 64], mmdt, kind="ExternalInput")
    bth = nc.dram_tensor("bth", [64, EPC, G], f32, kind="ExternalInput")
    out = nc.dram_tensor("out", [EPC, G, 64, cap], f32, kind="ExternalOutput")

    relu = mybir.ActivationFunctionType.Relu
    mx = mybir.AluOpType.max
    add = mybir.AluOpType.add
    NIT = EPC * G

    with tile.TileContext(nc) as tc:
        with (
            tc.tile_pool(name="singles", bufs=1) as singles,
            tc.tile_pool(name="xin", bufs=6) as xin,
            tc.tile_pool(name="tin", bufs=6) as tin,
            tc.tile_pool(name="sb", bufs=4) as sb,
            tc.tile_pool(name="ob", bufs=4) as ob,
            tc.tile_pool(name="qp", bufs=6, space="PSUM") as qp,
            tc.tile_pool(name="dp", bufs=2, space="PSUM") as dpool,
        ):
            wE = singles.tile([128, EPC, G, 8], mmdt)
            nc.gpsimd.dma_start(out=wE[:], in_=weh[:])
            bE = singles.tile([8, EPC, G], f32)
            nc.gpsimd.dma_start(out=bE[:], in_=beh[:])
            wD = singles.tile([8, EPC, G, 128], mmdt)
            nc.gpsimd.dma_start(out=wD[:], in_=wdh[:])
            bD = singles.tile([128, EPC, G], f32)
            nc.gpsimd.dma_start(out=bD[:], in_=bdh[:])
            tD = singles.tile([128, EPC, G, 64], mmdt)
            nc.gpsimd.dma_start(out=tD[:], in_=tdh[:])
            bT = singles.tile([64, EPC, G], f32)
            nc.gpsimd.dma_start(out=bT[:], in_=bth[:])

            st = {}  # per-iteration live tiles

            def eg(i):
                return i // G, i % G

            # software pipeline: at step i emit conv(i), enc(i-1), dec(i-2),
            # tconv(i-3) so every PE instruction's inputs are one full
            # iteration old -- PE never stalls on same-iteration epilogues.
            for i in range(NIT + 3):
                if i < NIT:
                    e, g = eg(i)
                    xg = xin.tile([64, cap], mmdt, tag="xg")
                    nc.sync.dma_start(out=xg[:], in_=xt[:, e, g, :])
                    t4 = tin.tile([64, 4, 128], mmdt, tag="t4")
                    nc.sync.dma_start(out=t4[:], in_=th[:, e, g])
                    qs = []
                    for ph in range(4):
                        q = qp.tile([128, cap], f32, tag="q")
                        nc.tensor.matmul(
                            q[:], t4[:, ph, :], xg[:], start=True, stop=True
                        )
                        qs.append(q)
                    st[i] = {"qs": qs}

                if i - 1 >= 0 and i - 1 < NIT:
                    j = i - 1
                    e, g = eg(j)
                    zpt = dpool.tile([8, cap], f32, tag="dzyo")
                    nc.tensor.matmul(
                        zpt[:], wE[:, e, g, :], st[j]["feat"][:],
                        start=True, stop=True,
                    )
                    st[j]["zp"] = zpt

                if i - 2 >= 0 and i - 2 < NIT:
                    j = i - 2
                    e, g = eg(j)
                    ypt = dpool.tile([128, cap], f32, tag="dzyo")
                    nc.tensor.matmul(
                        ypt[:], wD[:, e, g, :], st[j]["z"][:],
                        start=True, stop=True,
                    )
                    st[j]["ypp"] = ypt

                if i - 3 >= 0:
                    j = i - 3
                    e, g = eg(j)
                    opt = dpool.tile([64, cap], f32, tag="dzyo")
                    nc.tensor.matmul(
                        opt[:], tD[:, e, g, :], st[j]["y"][:],
                        start=True, stop=True,
                    )
                    o = ob.tile([64, cap], f32, tag="os")
                    nc.vector.tensor_scalar(
                        o[:], opt[:], bT[:, e, g : g + 1], 0.0, add, mx
                    )
                    nc.scalar.dma_start(out=out[e, g], in_=o[:])
                    del st[j]

                # epilogues (emitted after the PE block so engine streams
                # pick them up while PE moves on)
                if i < NIT:
                    qs = st[i]["qs"]
                    s0 = sb.tile([128, cap], f32, tag="s0")
                    nc.scalar.activation(s0[:], qs[0][:], relu)
                    m01 = sb.tile([128, cap], f32, tag="m01")
                    nc.vector.scalar_tensor_tensor(
                        m01[:], qs[1][:], 0.0, s0[:], op0=mx, op1=mx
                    )
                    m012 = sb.tile([128, cap], f32, tag="m012")
                    nc.vector.scalar_tensor_tensor(
                        m012[:], qs[2][:], 0.0, m01[:], op0=mx, op1=mx
                    )
                    feat = sb.tile([128, cap], mmdt, tag="feat")
                    nc.vector.scalar_tensor_tensor(
                        feat[:], qs[3][:], 0.0, m012[:], op0=mx, op1=mx
                    )
                    st[i]["feat"] = feat

                if i - 1 >= 0 and i - 1 < NIT:
                    j = i - 1
                    e, g = eg(j)
                    z = sb.tile([8, cap], mmdt, tag="zs")
                    nc.scalar.activation(
                        z[:], st[j]["zp"][:], relu, bias=bE[:, e, g : g + 1]
                    )
                    st[j]["z"] = z

                if i - 2 >= 0 and i - 2 < NIT:
                    j = i - 2
                    e, g = eg(j)
                    y = sb.tile([128, cap], mmdt, tag="ys")
                    nc.scalar.activation(
                        y[:], st[j]["ypp"][:], relu, bias=bD[:, e, g : g + 1]
                    )
                    st[j]["y"] = y

    nc.compile()
    return nc


# ---------------------------------------------------------------- numpy fallback
def _numpy_fallback(x, keys, t_enc, t_dec, edw, edb, ddw, ddb, dtb):
    n = x.shape[0]
    xf = x.reshape(n, G, 64)
    res = np.zeros((n, G, 64), np.float32)
    for e in range(E):
        idx = np.where(keys == e)[0]
        if len(idx) == 0:
            continue
        for g in range(G):
            xg = xf[idx, g].T
            q = [t_enc[e, g, ph].T @ xg for ph in range(4)]
            feat = np.maximum(
                np.maximum(np.maximum(q[0], q[1]), np.maximum(q[2], q[3])), 0.0
            )
            z = np.maximum(edw[e, g] @ feat + edb[e, g][:, None], 0.0)
            y = np.maximum(ddw[e, g] @ z + ddb[e, g][:, None], 0.0)
            o = np.maximum(t_dec[e, g].T @ y + dtb[e, g], 0.0)
            res[idx, g] = o.T
    return res.reshape(n, G, 8, 8)


# ---------------------------------------------------------------- entry point
def kernel(
    x,
    keys,
    conv_mask,
    enc_conv_w,
    enc_dense_w,
    enc_dense_b,
    dec_dense_w,
    dec_dense_b,
    dec_tconv_w,
    dec_tconv_b,
):
    x = np.asarray(x, np.float32)
    keys_np = np.asarray(keys).astype(np.int64)
    conv_mask = np.asarray(conv_mask, np.float32)
    enc_conv_w = np.asarray(enc_conv_w, np.float32)
    edw = np.asarray(enc_dense_w, np.float32)
    edb = np.asarray(enc_dense_b, np.float32)
    ddw = np.asarray(dec_dense_w, np.float32)
    ddb = np.asarray(dec_dense_b, np.float32)
    dtw = np.asarray(dec_tconv_w, np.float32)
    dtb = np.asarray(dec_tconv_b, np.float32)

    n = x.shape[0]
    t_enc = _build_t_enc(conv_mask, enc_conv_w)
    t_dec = _build_t_dec(dtw)

    counts = np.bincount(keys_np, minlength=E)
    cap = int(-(-max(counts.max(), 16) // 16) * 16)
    if cap > 512:
        return _numpy_fallback(x, keys_np, t_enc, t_dec, edw, edb, ddw, ddb, dtb)

    order = np.argsort(keys_np, kind="stable")
    starts = np.concatenate([[0], np.cumsum(counts)])
    xs = x.reshape(n, G, 64)[order]  # sorted by expert

    # bucketed input, conv-matmul layout [pix, e, g, slot]
    xh = np.zeros((E, G, 64, cap), np.float32)
    for e in range(E):
        ne = counts[e]
        if ne:
            xh[e, :, :, :ne] = xs[starts[e] : starts[e + 1]].transpose(1, 2, 0)

    # global weight hosts in exact SBUF layouts (partition dim first)
    mmnp = np.float16 if MM_DTYPE == "float16" else np.float32
    thg = np.ascontiguousarray(t_enc.transpose(3, 0, 1, 2, 4), mmnp)  # [64,E,G,4,128]
    wehg = np.ascontiguousarray(edw.transpose(3, 0, 1, 2), mmnp)  # [128,E,G,8]
    behg = np.ascontiguousarray(edb.transpose(2, 0, 1))  # [8,E,G]
    wdhg = np.ascontiguousarray(ddw.transpose(3, 0, 1, 2), mmnp)  # [8,E,G,128]
    bdhg = np.ascontiguousarray(ddb.transpose(2, 0, 1))  # [128,E,G]
    tdhg = np.ascontiguousarray(t_dec.transpose(2, 0, 1, 3), mmnp)  # [128,E,G,64]
    bthg = np.ascontiguousarray(
        np.broadcast_to(dtb[None, :, :], (64, E, G))
    )  # [64,E,G]
    xhg = np.ascontiguousarray(xh.transpose(2, 0, 1, 3), mmnp)  # [64,E,G,cap]

    cache_key = (cap, MM_DTYPE)
    if cache_key not in _PROG_CACHE:
        _PROG_CACHE[cache_key] = _build_program(cap)
    nc = _PROG_CACHE[cache_key]

    def core_slice(a, c):
        return np.ascontiguousarray(a[:, EPC * c : EPC * (c + 1)])

    in_maps = [
        {
            "xt": core_slice(xhg, c),
            "th": core_slice(thg, c),
            "weh": core_slice(wehg, c),
            "beh": core_slice(behg, c),
            "wdh": core_slice(wdhg, c),
            "bdh": core_slice(bdhg, c),
            "tdh": core_slice(tdhg, c),
            "bth": core_slice(bthg, c),
        }
        for c in range(N_CORES)
    ]

    global _LAST_IN_MAPS
    _LAST_IN_MAPS = in_maps

    from concourse.bass_utils import run_bass_kernel_spmd

    res = run_bass_kernel_spmd(nc, in_maps, core_ids=list(range(N_CORES)))

    og = np.concatenate([res.results[c]["out"] for c in range(N_CORES)], axis=0)
    result = np.empty((n, G, 64), np.float32)
    for e in range(E):
        ne = counts[e]
        if ne:
            result[order[starts[e] : starts[e + 1]]] = og[e, :, :, :ne].transpose(
                2, 0, 1
            )
    return result.reshape(n, G, 8, 8)



# revision 11
# speedup vs baseline: 1.0259x; 1.0259x over previous
"""Trainium2 Bass kernel for nn_AutoEncoderModular (MoE-routed grouped autoencoder).

Strategy: expert-parallel. Host sorts wafers by expert key into 16
capacity-padded buckets; core c handles experts (2c, 2c+1). Every stage is a
matmul with wafers in the moving/free dimension:

  - encoder conv (masked 3x3, groups=14, 1->8ch) is decomposed into 4 matmuls
    per (expert, group), one per 2x2 maxpool phase, with a host-built
    Toeplitz-x-weight stationary [64 in-pix, 128=(8ch x 16 pooled-pix)].
    ReLU commutes with max, so maxpool = elementwise max of the 4 phase tiles,
    which lands `feat` directly in the [128 features, wafers] layout the dense
    encoder needs (no transposes anywhere).
  - enc dense K=128->M=8, dec dense K=8->M=128, and the stride-2 grouped
    ConvTranspose as a host-built [K=128, M=64] linear map; bias+ReLU fused
    into ACT/DVE epilogue ops.
"""

import numpy as np

E, G, CPG = 16, 14, 8
N_CORES = 8
EPC = E // N_CORES  # experts per core

MM_DTYPE = "float16"  # "float32r" (TF32-like) or "float32" (exact)
_PROG_CACHE = {}
_LAST_IN_MAPS = None


# ---------------------------------------------------------------- tables
def _build_t_enc(conv_mask, enc_conv_w):
    """[E, G, 4, 64, 128] : phase (yb,xb); col m = co*16 + y2*4 + x2."""
    wm = (enc_conv_w[:, :, 0] * conv_mask).reshape(E, G, CPG, 3, 3)
    t = np.zeros((E, G, 4, 64, 128), np.float32)
    for ph in range(4):
        yb, xb = ph // 2, ph % 2
        for dy in range(3):
            for dx in range(3):
                if conv_mask[dy, dx] == 0.0:
                    continue
                for y2 in range(4):
                    yi = 2 * y2 + yb + dy - 1
                    if not (0 <= yi < 8):
                        continue
                    for x2 in range(4):
                        xi = 2 * x2 + xb + dx - 1
                        if not (0 <= xi < 8):
                            continue
                        pi = yi * 8 + xi
                        for co in range(CPG):
                            m = co * 16 + y2 * 4 + x2
                            t[:, :, ph, pi, m] += wm[:, :, co, dy, dx]
    return t


def _build_t_dec(dec_tconv_w):
    """[E, G, 128, 64] : ConvTranspose2d(s=2, p=1, op=1) as a linear map.
    in f = c*16 + yi*4 + xi ; out po = yo*8 + xo ; yo = 2*yi + ky - 1."""
    w = dec_tconv_w[:, :, 0].reshape(E, G, CPG, 3, 3)
    t = np.zeros((E, G, 128, 64), np.float32)
    for c in range(CPG):
        for yi in range(4):
            for xi in range(4):
                f = c * 16 + yi * 4 + xi
                for ky in range(3):
                    yo = 2 * yi + ky - 1
                    if not (0 <= yo < 8):
                        continue
                    for kx in range(3):
                        xo = 2 * xi + kx - 1
                        if not (0 <= xo < 8):
                            continue
                        t[:, :, f, yo * 8 + xo] += w[:, :, c, ky, kx]
    return t


# ---------------------------------------------------------------- device program
def _build_program(cap):
    import concourse.tile as tile
    from concourse import bacc, mybir

    f32 = mybir.dt.float32
    mmdt = getattr(mybir.dt, MM_DTYPE)
    nc = bacc.Bacc("TRN2", target_bir_lowering=False)

    xt = nc.dram_tensor("xt", [64, EPC, G, cap], mmdt, kind="ExternalInput")
    th = nc.dram_tensor("th", [64, EPC, G, 4, 128], mmdt, kind="ExternalInput")
    weh = nc.dram_tensor("weh", [128, EPC, G, 8], mmdt, kind="ExternalInput")
    beh = nc.dram_tensor("beh", [8, EPC, G], f32, kind="ExternalInput")
    wdh = nc.dram_tensor("wdh", [8, EPC, G, 128], mmdt, kind="ExternalInput")
    bdh = nc.dram_tensor("bdh", [128, EPC, G], f32, kind="ExternalInput")
    tdh = nc.dram_tensor("tdh", [128, EPC, G, 64], mmdt, kind="ExternalInput")
    bth = nc.dram_tensor("bth", [64, EPC, G], f32, kind="ExternalInput")
    out = nc.dram_tensor("out", [EPC, G, 64, cap], f32, kind="ExternalOutput")

    relu = mybir.ActivationFunctionType.Relu
    mx = mybir.AluOpType.max
    add = mybir.AluOpType.add
    NIT = EPC * G

    with tile.TileContext(nc) as tc:
        with (
            tc.tile_pool(name="singles", bufs=1) as singles,
            tc.tile_pool(name="xin", bufs=6) as xin,
            tc.tile_pool(name="tin", bufs=6) as tin,
            tc.tile_pool(name="sb", bufs=4) as sb,
            tc.tile_pool(name="ob", bufs=4) as ob,
            tc.tile_pool(name="qp", bufs=6, space="PSUM") as qp,
            tc.tile_pool(name="dp", bufs=2, space="PSUM") as dpool,
        ):
            wE = singles.tile([128, EPC, G, 8], mmdt)
            nc.gpsimd.dma_start(out=wE[:], in_=weh[:])
            bE = singles.tile([8, EPC, G], f32)
            nc.gpsimd.dma_start(out=bE[:], in_=beh[:])
            wD = singles.tile([8, EPC, G, 128], mmdt)
            nc.gpsimd.dma_start(out=wD[:], in_=wdh[:])
            bD = singles.tile([128, EPC, G], f32)
            nc.gpsimd.dma_start(out=bD[:], in_=bdh[:])
            tD = singles.tile([128, EPC, G, 64], mmdt)
            nc.gpsimd.dma_start(out=tD[:], in_=tdh[:])
            bT = singles.tile([64, EPC, G], f32)
            nc.gpsimd.dma_start(out=bT[:], in_=bth[:])

            st = {}  # per-iteration live tiles

            def eg(i):
                return i // G, i % G

            # software pipeline: at step i emit conv(i), enc(i-1), dec(i-2),
            # tconv(i-3) so every PE instruction's inputs are one full
            # iteration old -- PE never stalls on same-iteration epilogues.
            for i in range(NIT + 3):
                if i < NIT:
                    e, g = eg(i)
                    xg = xin.tile([64, cap], mmdt, tag="xg")
                    nc.sync.dma_start(out=xg[:], in_=xt[:, e, g, :])
                    t4 = tin.tile([64, 4, 128], mmdt, tag="t4")
                    nc.sync.dma_start(out=t4[:], in_=th[:, e, g])
                    qs = []
                    for ph in range(4):
                        q = qp.tile([128, cap], f32, tag="q")
                        nc.tensor.matmul(
                            q[:], t4[:, ph, :], xg[:], start=True, stop=True
                        )
                        qs.append(q)
                    st[i] = {"qs": qs}

                if i - 1 >= 0 and i - 1 < NIT:
                    j = i - 1
                    e, g = eg(j)
                    zpt = dpool.tile([8, cap], f32, tag="dzyo")
                    nc.tensor.matmul(
                        zpt[:], wE[:, e, g, :], st[j]["feat"][:],
                        start=True, stop=True,
                    )
                    st[j]["zp"] = zpt

                if i - 2 >= 0 and i - 2 < NIT:
                    j = i - 2
                    e, g = eg(j)
                    ypt = dpool.tile([128, cap], f32, tag="dzyo")
                    nc.tensor.matmul(
                        ypt[:], wD[:, e, g, :], st[j]["z"][:],
                        start=True, stop=True,
                    )
                    st[j]["ypp"] = ypt

                if i - 3 >= 0:
                    j = i - 3
                    e, g = eg(j)
                    opt = dpool.tile([64, cap], f32, tag="dzyo")
                    nc.tensor.matmul(
                        opt[:], tD[:, e, g, :], st[j]["y"][:],
                        start=True, stop=True,
                    )
                    o = ob.tile([64, cap], f32, tag="os")
                    nc.vector.tensor_scalar(
                        o[:], opt[:], bT[:, e, g : g + 1], 0.0, add, mx
                    )
                    nc.scalar.dma_start(out=out[e, g], in_=o[:])
                    del st[j]

                # epilogues (emitted after the PE block so engine streams
                # pick them up while PE moves on)
                if i < NIT:
                    qs = st[i]["qs"]
                    s0 = sb.tile([128, cap], f32, tag="s0")
                    nc.scalar.activation(s0[:], qs[0][:], relu)
                    m01 = sb.tile([128, cap], f32, tag="m01")
                    nc.vector.scalar_tensor_tensor(
                        m01[:], qs[1][:], 0.0, s0[:], op0=mx, op1=mx
                    )
                    m012 = sb.tile([128, cap], f32, tag="m012")
                    nc.vector.scalar_tensor_tensor(
                        m012[:], qs[2][:], 0.0, m01[:], op0=mx, op1=mx
                    )
                    feat = sb.tile([128, cap], mmdt, tag="feat")
                    nc.vector.scalar_tensor_tensor(
                        feat[:], qs[3][:], 0.0, m012[:], op0=mx, op1=mx
                    )
                    st[i]["feat"] = feat

                if i - 1 >= 0 and i - 1 < NIT:
                    j = i - 1
                    e, g = eg(j)
                    z = sb.tile([8, cap], mmdt, tag="zs")
                    nc.scalar.activation(
                        z[:], st[j]["zp"][:], relu, bias=bE[:, e, g : g + 1]
                    )
                    st[j]["z"] = z

                if i - 2 >= 0 and i - 2 < NIT:
                    j = i - 2
                    e, g = eg(j)
                    y = sb.tile([128, cap], mmdt, tag="ys")
                    nc.scalar.activation(
                        y[:], st[j]["ypp"][:], relu, bias=bD[:, e, g : g + 1]
                    )
                    st[j]["y"] = y

    nc.compile()
    return nc


# ---------------------------------------------------------------- numpy fallback
def _numpy_fallback(x, keys, t_enc, t_dec, edw, edb, ddw, ddb, dtb):
    n = x.shape[0]
    xf = x.reshape(n, G, 64)
    res = np.zeros((n, G, 64), np.float32)
    for e in range(E):
        idx = np.where(keys == e)[0]
        if len(idx) == 0:
            continue
        for g in range(G):
            xg = xf[idx, g].T
            q = [t_enc[e, g, ph].T @ xg for ph in range(4)]
            feat = np.maximum(
                np.maximum(np.maximum(q[0], q[1]), np.maximum(q[2], q[3])), 0.0
            )
            z = np.maximum(edw[e, g] @ feat + edb[e, g][:, None], 0.0)
            y = np.maximum(ddw[e, g] @ z + ddb[e, g][:, None], 0.0)
            o = np.maximum(t_dec[e, g].T @ y + dtb[e, g], 0.0)
            res[idx, g] = o.T
    return res.reshape(n, G, 8, 8)


# ---------------------------------------------------------------- entry point
def kernel(
    x,
    keys,
    conv_mask,
    enc_conv_w,
    enc_dense_w,
    enc_dense_b,
    dec_dense_w,
    dec_dense_b,
    dec_tconv_w,
    dec_tconv_b,
):
    x = np.asarray(x, np.float32)
    keys_np = np.asarray(keys).astype(np.int64)
    conv_mask = np.asarray(conv_mask, np.float32)
    enc_conv_w = np.asarray(enc_conv_w, np.float32)
    edw = np.asarray(enc_dense_w, np.float32)
    edb = np.asarray(enc_dense_b, np.float32)
    ddw = np.asarray(dec_dense_w, np.float32)
    ddb = np.asarray(dec_dense_b, np.float32)
    dtw = np.asarray(dec_tconv_w, np.float32)
    dtb = np.asarray(dec_tconv_b, np.float32)

    n = x.shape[0]
    t_enc = _build_t_enc(conv_mask, enc_conv_w)
    t_dec = _build_t_dec(dtw)

    counts = np.bincount(keys_np, minlength=E)
    cap = int(-(-max(counts.max(), 16) // 8) * 8)
    if cap > 512:
        return _numpy_fallback(x, keys_np, t_enc, t_dec, edw, edb, ddw, ddb, dtb)

    order = np.argsort(keys_np, kind="stable")
    starts = np.concatenate([[0], np.cumsum(counts)])
    xs = x.reshape(n, G, 64)[order]  # sorted by expert

    # bucketed input, conv-matmul layout [pix, e, g, slot]
    xh = np.zeros((E, G, 64, cap), np.float32)
    for e in range(E):
        ne = counts[e]
        if ne:
            xh[e, :, :, :ne] = xs[starts[e] : starts[e + 1]].transpose(1, 2, 0)

    # global weight hosts in exact SBUF layouts (partition dim first)
    mmnp = np.float16 if MM_DTYPE == "float16" else np.float32
    thg = np.ascontiguousarray(t_enc.transpose(3, 0, 1, 2, 4), mmnp)  # [64,E,G,4,128]
    wehg = np.ascontiguousarray(edw.transpose(3, 0, 1, 2), mmnp)  # [128,E,G,8]
    behg = np.ascontiguousarray(edb.transpose(2, 0, 1))  # [8,E,G]
    wdhg = np.ascontiguousarray(ddw.transpose(3, 0, 1, 2), mmnp)  # [8,E,G,128]
    bdhg = np.ascontiguousarray(ddb.transpose(2, 0, 1))  # [128,E,G]
    tdhg = np.ascontiguousarray(t_dec.transpose(2, 0, 1, 3), mmnp)  # [128,E,G,64]
    bthg = np.ascontiguousarray(
        np.broadcast_to(dtb[None, :, :], (64, E, G))
    )  # [64,E,G]
    xhg = np.ascontiguousarray(xh.transpose(2, 0, 1, 3), mmnp)  # [64,E,G,cap]

    cache_key = (cap, MM_DTYPE)
    if cache_key not in _PROG_CACHE:
        _PROG_CACHE[cache_key] = _build_program(cap)
    nc = _PROG_CACHE[cache_key]

    def core_slice(a, c):
        return np.ascontiguousarray(a[:, EPC * c : EPC * (c + 1)])

    in_maps = [
        {
            "xt": core_slice(xhg, c),
            "th": core_slice(thg, c),
            "weh": core_slice(wehg, c),
            "beh": core_slice(behg, c),
            "wdh": core_slice(wdhg, c),
            "bdh": core_slice(bdhg, c),
            "tdh": core_slice(tdhg, c),
            "bth": core_slice(bthg, c),
        }
        for c in range(N_CORES)
    ]

    global _LAST_IN_MAPS
    _LAST_IN_MAPS = in_maps

    from concourse.bass_utils import run_bass_kernel_spmd

    res = run_bass_kernel_spmd(nc, in_maps, core_ids=list(range(N_CORES)))

    og = np.concatenate([res.results[c]["out"] for c in range(N_CORES)], axis=0)
    result = np.empty((n, G, 64), np.float32)
    for e in range(E):
        ne = counts[e]
        if ne:
            result[order[starts[e] : starts[e + 1]]] = og[e, :, :, :ne].transpose(
                2, 0, 1
            )
    return result.reshape(n, G, 8, 8)



# revision 12
# speedup vs baseline: 1.0338x; 1.0077x over previous
"""Trainium2 Bass kernel for nn_AutoEncoderModular (MoE-routed grouped autoencoder).

Strategy: expert-parallel. Host sorts wafers by expert key into 16
capacity-padded buckets; core c handles experts (2c, 2c+1). Every stage is a
matmul with wafers in the moving/free dimension:

  - encoder conv (masked 3x3, groups=14, 1->8ch) is decomposed into 4 matmuls
    per (expert, group), one per 2x2 maxpool phase, with a host-built
    Toeplitz-x-weight stationary [64 in-pix, 128=(8ch x 16 pooled-pix)].
    ReLU commutes with max, so maxpool = elementwise max of the 4 phase tiles,
    which lands `feat` directly in the [128 features, wafers] layout the dense
    encoder needs (no transposes anywhere).
  - enc dense K=128->M=8, dec dense K=8->M=128, and the stride-2 grouped
    ConvTranspose as a host-built [K=128, M=64] linear map; bias+ReLU fused
    into ACT/DVE epilogue ops.
"""

import numpy as np

E, G, CPG = 16, 14, 8
N_CORES = 8
EPC = E // N_CORES  # experts per core

MM_DTYPE = "float16"  # "float32r" (TF32-like) or "float32" (exact)
_PROG_CACHE = {}
_LAST_IN_MAPS = None


# ---------------------------------------------------------------- tables
def _build_t_enc(conv_mask, enc_conv_w):
    """[E, G, 4, 64, 128] : phase (yb,xb); col m = co*16 + y2*4 + x2."""
    wm = (enc_conv_w[:, :, 0] * conv_mask).reshape(E, G, CPG, 3, 3)
    t = np.zeros((E, G, 4, 64, 128), np.float32)
    for ph in range(4):
        yb, xb = ph // 2, ph % 2
        for dy in range(3):
            for dx in range(3):
                if conv_mask[dy, dx] == 0.0:
                    continue
                for y2 in range(4):
                    yi = 2 * y2 + yb + dy - 1
                    if not (0 <= yi < 8):
                        continue
                    for x2 in range(4):
                        xi = 2 * x2 + xb + dx - 1
                        if not (0 <= xi < 8):
                            continue
                        pi = yi * 8 + xi
                        for co in range(CPG):
                            m = co * 16 + y2 * 4 + x2
                            t[:, :, ph, pi, m] += wm[:, :, co, dy, dx]
    return t


def _build_t_dec(dec_tconv_w):
    """[E, G, 128, 64] : ConvTranspose2d(s=2, p=1, op=1) as a linear map.
    in f = c*16 + yi*4 + xi ; out po = yo*8 + xo ; yo = 2*yi + ky - 1."""
    w = dec_tconv_w[:, :, 0].reshape(E, G, CPG, 3, 3)
    t = np.zeros((E, G, 128, 64), np.float32)
    for c in range(CPG):
        for yi in range(4):
            for xi in range(4):
                f = c * 16 + yi * 4 + xi
                for ky in range(3):
                    yo = 2 * yi + ky - 1
                    if not (0 <= yo < 8):
                        continue
                    for kx in range(3):
                        xo = 2 * xi + kx - 1
                        if not (0 <= xo < 8):
                            continue
                        t[:, :, f, yo * 8 + xo] += w[:, :, c, ky, kx]
    return t


# ---------------------------------------------------------------- device program
def _build_program(cap):
    import concourse.tile as tile
    from concourse import bacc, mybir

    f32 = mybir.dt.float32
    mmdt = getattr(mybir.dt, MM_DTYPE)
    nc = bacc.Bacc("TRN2", target_bir_lowering=False)

    xt = nc.dram_tensor("xt", [64, EPC, G, cap], mmdt, kind="ExternalInput")
    th = nc.dram_tensor("th", [64, EPC, G, 4, 128], mmdt, kind="ExternalInput")
    weh = nc.dram_tensor("weh", [128, EPC, G, 8], mmdt, kind="ExternalInput")
    beh = nc.dram_tensor("beh", [8, EPC, G], f32, kind="ExternalInput")
    wdh = nc.dram_tensor("wdh", [8, EPC, G, 128], mmdt, kind="ExternalInput")
    bdh = nc.dram_tensor("bdh", [128, EPC, G], f32, kind="ExternalInput")
    tdh = nc.dram_tensor("tdh", [128, EPC, G, 64], mmdt, kind="ExternalInput")
    bth = nc.dram_tensor("bth", [64, EPC, G], f32, kind="ExternalInput")
    out = nc.dram_tensor("out", [EPC, G, 64, cap], f32, kind="ExternalOutput")

    relu = mybir.ActivationFunctionType.Relu
    mx = mybir.AluOpType.max
    add = mybir.AluOpType.add
    NIT = EPC * G

    with tile.TileContext(nc) as tc:
        with (
            tc.tile_pool(name="singles", bufs=1) as singles,
            tc.tile_pool(name="xin", bufs=6) as xin,
            tc.tile_pool(name="tin", bufs=6) as tin,
            tc.tile_pool(name="sb", bufs=4) as sb,
            tc.tile_pool(name="ob", bufs=4) as ob,
            tc.tile_pool(name="qp", bufs=6, space="PSUM") as qp,
            tc.tile_pool(name="dp", bufs=2, space="PSUM") as dpool,
        ):
            # prefetch iteration 0 inputs on the earliest-live queue
            xg0 = xin.tile([64, cap], mmdt, tag="xg")
            nc.gpsimd.dma_start(out=xg0[:], in_=xt[:, 0, 0, :])
            t40 = tin.tile([64, 4, 128], mmdt, tag="t4")
            nc.gpsimd.dma_start(out=t40[:], in_=th[:, 0, 0])

            wE = singles.tile([128, EPC, G, 8], mmdt)
            nc.gpsimd.dma_start(out=wE[:], in_=weh[:])
            bE = singles.tile([8, EPC, G], f32)
            nc.gpsimd.dma_start(out=bE[:], in_=beh[:])
            wD = singles.tile([8, EPC, G, 128], mmdt)
            nc.gpsimd.dma_start(out=wD[:], in_=wdh[:])
            bD = singles.tile([128, EPC, G], f32)
            nc.gpsimd.dma_start(out=bD[:], in_=bdh[:])
            tD = singles.tile([128, EPC, G, 64], mmdt)
            nc.gpsimd.dma_start(out=tD[:], in_=tdh[:])
            bT = singles.tile([64, EPC, G], f32)
            nc.gpsimd.dma_start(out=bT[:], in_=bth[:])

            st = {}  # per-iteration live tiles

            def eg(i):
                return i // G, i % G

            # software pipeline: at step i emit conv(i), enc(i-1), dec(i-2),
            # tconv(i-3) so every PE instruction's inputs are one full
            # iteration old -- PE never stalls on same-iteration epilogues.
            for i in range(NIT + 3):
                if i < NIT:
                    e, g = eg(i)
                    if i == 0:
                        xg, t4 = xg0, t40
                    else:
                        xg = xin.tile([64, cap], mmdt, tag="xg")
                        nc.sync.dma_start(out=xg[:], in_=xt[:, e, g, :])
                        t4 = tin.tile([64, 4, 128], mmdt, tag="t4")
                        nc.sync.dma_start(out=t4[:], in_=th[:, e, g])
                    qs = []
                    for ph in range(4):
                        q = qp.tile([128, cap], f32, tag="q")
                        nc.tensor.matmul(
                            q[:], t4[:, ph, :], xg[:], start=True, stop=True
                        )
                        qs.append(q)
                    st[i] = {"qs": qs}

                if i - 1 >= 0 and i - 1 < NIT:
                    j = i - 1
                    e, g = eg(j)
                    zpt = dpool.tile([8, cap], f32, tag="dzyo")
                    nc.tensor.matmul(
                        zpt[:], wE[:, e, g, :], st[j]["feat"][:],
                        start=True, stop=True,
                    )
                    st[j]["zp"] = zpt

                if i - 2 >= 0 and i - 2 < NIT:
                    j = i - 2
                    e, g = eg(j)
                    ypt = dpool.tile([128, cap], f32, tag="dzyo")
                    nc.tensor.matmul(
                        ypt[:], wD[:, e, g, :], st[j]["z"][:],
                        start=True, stop=True,
                    )
                    st[j]["ypp"] = ypt

                if i - 3 >= 0:
                    j = i - 3
                    e, g = eg(j)
                    opt = dpool.tile([64, cap], f32, tag="dzyo")
                    nc.tensor.matmul(
                        opt[:], tD[:, e, g, :], st[j]["y"][:],
                        start=True, stop=True,
                    )
                    o = ob.tile([64, cap], f32, tag="os")
                    nc.vector.tensor_scalar(
                        o[:], opt[:], bT[:, e, g : g + 1], 0.0, add, mx
                    )
                    nc.scalar.dma_start(out=out[e, g], in_=o[:])
                    del st[j]

                # epilogues (emitted after the PE block so engine streams
                # pick them up while PE moves on)
                if i < NIT:
                    qs = st[i]["qs"]
                    s0 = sb.tile([128, cap], f32, tag="s0")
                    nc.scalar.activation(s0[:], qs[0][:], relu)
                    m01 = sb.tile([128, cap], f32, tag="m01")
                    nc.vector.scalar_tensor_tensor(
                        m01[:], qs[1][:], 0.0, s0[:], op0=mx, op1=mx
                    )
                    m012 = sb.tile([128, cap], f32, tag="m012")
                    nc.vector.scalar_tensor_tensor(
                        m012[:], qs[2][:], 0.0, m01[:], op0=mx, op1=mx
                    )
                    feat = sb.tile([128, cap], mmdt, tag="feat")
                    nc.vector.scalar_tensor_tensor(
                        feat[:], qs[3][:], 0.0, m012[:], op0=mx, op1=mx
                    )
                    st[i]["feat"] = feat

                if i - 1 >= 0 and i - 1 < NIT:
                    j = i - 1
                    e, g = eg(j)
                    z = sb.tile([8, cap], mmdt, tag="zs")
                    nc.scalar.activation(
                        z[:], st[j]["zp"][:], relu, bias=bE[:, e, g : g + 1]
                    )
                    st[j]["z"] = z

                if i - 2 >= 0 and i - 2 < NIT:
                    j = i - 2
                    e, g = eg(j)
                    y = sb.tile([128, cap], mmdt, tag="ys")
                    nc.scalar.activation(
                        y[:], st[j]["ypp"][:], relu, bias=bD[:, e, g : g + 1]
                    )
                    st[j]["y"] = y

    nc.compile()
    return nc


# ---------------------------------------------------------------- numpy fallback
def _numpy_fallback(x, keys, t_enc, t_dec, edw, edb, ddw, ddb, dtb):
    n = x.shape[0]
    xf = x.reshape(n, G, 64)
    res = np.zeros((n, G, 64), np.float32)
    for e in range(E):
        idx = np.where(keys == e)[0]
        if len(idx) == 0:
            continue
        for g in range(G):
            xg = xf[idx, g].T
            q = [t_enc[e, g, ph].T @ xg for ph in range(4)]
            feat = np.maximum(
                np.maximum(np.maximum(q[0], q[1]), np.maximum(q[2], q[3])), 0.0
            )
            z = np.maximum(edw[e, g] @ feat + edb[e, g][:, None], 0.0)
            y = np.maximum(ddw[e, g] @ z + ddb[e, g][:, None], 0.0)
            o = np.maximum(t_dec[e, g].T @ y + dtb[e, g], 0.0)
            res[idx, g] = o.T
    return res.reshape(n, G, 8, 8)


# ---------------------------------------------------------------- entry point
def kernel(
    x,
    keys,
    conv_mask,
    enc_conv_w,
    enc_dense_w,
    enc_dense_b,
    dec_dense_w,
    dec_dense_b,
    dec_tconv_w,
    dec_tconv_b,
):
    x = np.asarray(x, np.float32)
    keys_np = np.asarray(keys).astype(np.int64)
    conv_mask = np.asarray(conv_mask, np.float32)
    enc_conv_w = np.asarray(enc_conv_w, np.float32)
    edw = np.asarray(enc_dense_w, np.float32)
    edb = np.asarray(enc_dense_b, np.float32)
    ddw = np.asarray(dec_dense_w, np.float32)
    ddb = np.asarray(dec_dense_b, np.float32)
    dtw = np.asarray(dec_tconv_w, np.float32)
    dtb = np.asarray(dec_tconv_b, np.float32)

    n = x.shape[0]
    t_enc = _build_t_enc(conv_mask, enc_conv_w)
    t_dec = _build_t_dec(dtw)

    counts = np.bincount(keys_np, minlength=E)
    cap = int(-(-max(counts.max(), 16) // 8) * 8)
    if cap > 512:
        return _numpy_fallback(x, keys_np, t_enc, t_dec, edw, edb, ddw, ddb, dtb)

    order = np.argsort(keys_np, kind="stable")
    starts = np.concatenate([[0], np.cumsum(counts)])
    xs = x.reshape(n, G, 64)[order]  # sorted by expert

    # bucketed input, conv-matmul layout [pix, e, g, slot]
    xh = np.zeros((E, G, 64, cap), np.float32)
    for e in range(E):
        ne = counts[e]
        if ne:
            xh[e, :, :, :ne] = xs[starts[e] : starts[e + 1]].transpose(1, 2, 0)

    # global weight hosts in exact SBUF layouts (partition dim first)
    mmnp = np.float16 if MM_DTYPE == "float16" else np.float32
    thg = np.ascontiguousarray(t_enc.transpose(3, 0, 1, 2, 4), mmnp)  # [64,E,G,4,128]
    wehg = np.ascontiguousarray(edw.transpose(3, 0, 1, 2), mmnp)  # [128,E,G,8]
    behg = np.ascontiguousarray(edb.transpose(2, 0, 1))  # [8,E,G]
    wdhg = np.ascontiguousarray(ddw.transpose(3, 0, 1, 2), mmnp)  # [8,E,G,128]
    bdhg = np.ascontiguousarray(ddb.transpose(2, 0, 1))  # [128,E,G]
    tdhg = np.ascontiguousarray(t_dec.transpose(2, 0, 1, 3), mmnp)  # [128,E,G,64]
    bthg = np.ascontiguousarray(
        np.broadcast_to(dtb[None, :, :], (64, E, G))
    )  # [64,E,G]
    xhg = np.ascontiguousarray(xh.transpose(2, 0, 1, 3), mmnp)  # [64,E,G,cap]

    cache_key = (cap, MM_DTYPE)
    if cache_key not in _PROG_CACHE:
        _PROG_CACHE[cache_key] = _build_program(cap)
    nc = _PROG_CACHE[cache_key]

    def core_slice(a, c):
        return np.ascontiguousarray(a[:, EPC * c : EPC * (c + 1)])

    in_maps = [
        {
            "xt": core_slice(xhg, c),
            "th": core_slice(thg, c),
            "weh": core_slice(wehg, c),
            "beh": core_slice(behg, c),
            "wdh": core_slice(wdhg, c),
            "bdh": core_slice(bdhg, c),
            "tdh": core_slice(tdhg, c),
            "bth": core_slice(bthg, c),
        }
        for c in range(N_CORES)
    ]

    global _LAST_IN_MAPS
    _LAST_IN_MAPS = in_maps

    from concourse.bass_utils import run_bass_kernel_spmd

    res = run_bass_kernel_spmd(nc, in_maps, core_ids=list(range(N_CORES)))

    og = np.concatenate([res.results[c]["out"] for c in range(N_CORES)], axis=0)
    result = np.empty((n, G, 64), np.float32)
    for e in range(E):
        ne = counts[e]
        if ne:
            result[order[starts[e] : starts[e + 1]]] = og[e, :, :, :ne].transpose(
                2, 0, 1
            )
    return result.reshape(n, G, 8, 8)



# revision 13
# speedup vs baseline: 1.0654x; 1.0306x over previous
"""Trainium2 Bass kernel for nn_AutoEncoderModular (MoE-routed grouped autoencoder).

Strategy: expert-parallel. Host sorts wafers by expert key into 16
capacity-padded buckets; core c handles experts (2c, 2c+1). Every stage is a
matmul with wafers in the moving/free dimension:

  - encoder conv (masked 3x3, groups=14, 1->8ch) is decomposed into 4 matmuls
    per (expert, group), one per 2x2 maxpool phase, with a host-built
    Toeplitz-x-weight stationary [64 in-pix, 128=(8ch x 16 pooled-pix)].
    ReLU commutes with max, so maxpool = elementwise max of the 4 phase tiles,
    which lands `feat` directly in the [128 features, wafers] layout the dense
    encoder needs (no transposes anywhere).
  - enc dense K=128->M=8, dec dense K=8->M=128, and the stride-2 grouped
    ConvTranspose as a host-built [K=128, M=64] linear map; bias+ReLU fused
    into ACT/DVE epilogue ops.
"""

import numpy as np

E, G, CPG = 16, 14, 8
N_CORES = 8
EPC = E // N_CORES  # experts per core

MM_DTYPE = "float16"  # "float32r" (TF32-like) or "float32" (exact)
_PROG_CACHE = {}
_LAST_IN_MAPS = None


# ---------------------------------------------------------------- tables
def _build_t_enc(conv_mask, enc_conv_w):
    """[E, G, 4, 64, 128] : phase (yb,xb); col m = co*16 + y2*4 + x2."""
    wm = (enc_conv_w[:, :, 0] * conv_mask).reshape(E, G, CPG, 3, 3)
    t = np.zeros((E, G, 4, 64, 128), np.float32)
    for ph in range(4):
        yb, xb = ph // 2, ph % 2
        for dy in range(3):
            for dx in range(3):
                if conv_mask[dy, dx] == 0.0:
                    continue
                for y2 in range(4):
                    yi = 2 * y2 + yb + dy - 1
                    if not (0 <= yi < 8):
                        continue
                    for x2 in range(4):
                        xi = 2 * x2 + xb + dx - 1
                        if not (0 <= xi < 8):
                            continue
                        pi = yi * 8 + xi
                        for co in range(CPG):
                            m = co * 16 + y2 * 4 + x2
                            t[:, :, ph, pi, m] += wm[:, :, co, dy, dx]
    return t


def _build_t_dec(dec_tconv_w):
    """[E, G, 128, 64] : ConvTranspose2d(s=2, p=1, op=1) as a linear map.
    in f = c*16 + yi*4 + xi ; out po = yo*8 + xo ; yo = 2*yi + ky - 1."""
    w = dec_tconv_w[:, :, 0].reshape(E, G, CPG, 3, 3)
    t = np.zeros((E, G, 128, 64), np.float32)
    for c in range(CPG):
        for yi in range(4):
            for xi in range(4):
                f = c * 16 + yi * 4 + xi
                for ky in range(3):
                    yo = 2 * yi + ky - 1
                    if not (0 <= yo < 8):
                        continue
                    for kx in range(3):
                        xo = 2 * xi + kx - 1
                        if not (0 <= xo < 8):
                            continue
                        t[:, :, f, yo * 8 + xo] += w[:, :, c, ky, kx]
    return t


# ---------------------------------------------------------------- device program
def _build_program(cap):
    import concourse.tile as tile
    from concourse import bacc, mybir

    f32 = mybir.dt.float32
    mmdt = getattr(mybir.dt, MM_DTYPE)
    nc = bacc.Bacc("TRN2", target_bir_lowering=False)

    xt = nc.dram_tensor("xt", [64, EPC, G, cap], mmdt, kind="ExternalInput")
    th = nc.dram_tensor("th", [64, EPC, G, 4, 128], mmdt, kind="ExternalInput")
    weh = nc.dram_tensor("weh", [128, EPC, G, 8], mmdt, kind="ExternalInput")
    beh = nc.dram_tensor("beh", [8, EPC, G], f32, kind="ExternalInput")
    wdh = nc.dram_tensor("wdh", [8, EPC, G, 128], mmdt, kind="ExternalInput")
    bdh = nc.dram_tensor("bdh", [128, EPC, G], f32, kind="ExternalInput")
    tdh = nc.dram_tensor("tdh", [128, EPC, G, 64], mmdt, kind="ExternalInput")
    bth = nc.dram_tensor("bth", [64, EPC, G], f32, kind="ExternalInput")
    out = nc.dram_tensor("out", [EPC, G, 64, cap], f32, kind="ExternalOutput")

    relu = mybir.ActivationFunctionType.Relu
    mx = mybir.AluOpType.max
    add = mybir.AluOpType.add
    NIT = EPC * G

    with tile.TileContext(nc) as tc:
        with (
            tc.tile_pool(name="singles", bufs=1) as singles,
            tc.tile_pool(name="xin", bufs=6) as xin,
            tc.tile_pool(name="tin", bufs=6) as tin,
            tc.tile_pool(name="sb", bufs=4) as sb,
            tc.tile_pool(name="ob", bufs=4) as ob,
            tc.tile_pool(name="qp", bufs=6, space="PSUM") as qp,
            tc.tile_pool(name="dp", bufs=2, space="PSUM") as dpool,
        ):
            # prefetch the first iterations' inputs on the earliest-live queue
            pre = {}
            for k in range(3):
                e0, g0 = k // G, k % G
                xgk = xin.tile([64, cap], mmdt, tag="xg", name=f"xgpre{k}")
                nc.gpsimd.dma_start(out=xgk[:], in_=xt[:, e0, g0, :])
                t4k = tin.tile([64, 4, 128], mmdt, tag="t4", name=f"t4pre{k}")
                nc.gpsimd.dma_start(out=t4k[:], in_=th[:, e0, g0])
                pre[k] = (xgk, t4k)

            wE = singles.tile([128, EPC, G, 8], mmdt)
            nc.gpsimd.dma_start(out=wE[:], in_=weh[:])
            bE = singles.tile([8, EPC, G], f32)
            nc.gpsimd.dma_start(out=bE[:], in_=beh[:])
            wD = singles.tile([8, EPC, G, 128], mmdt)
            nc.gpsimd.dma_start(out=wD[:], in_=wdh[:])
            bD = singles.tile([128, EPC, G], f32)
            nc.gpsimd.dma_start(out=bD[:], in_=bdh[:])
            tD = singles.tile([128, EPC, G, 64], mmdt)
            nc.gpsimd.dma_start(out=tD[:], in_=tdh[:])
            bT = singles.tile([64, EPC, G], f32)
            nc.gpsimd.dma_start(out=bT[:], in_=bth[:])

            st = {}  # per-iteration live tiles

            def eg(i):
                return i // G, i % G

            # software pipeline: at step i emit conv(i), enc(i-1), dec(i-2),
            # tconv(i-3) so every PE instruction's inputs are one full
            # iteration old -- PE never stalls on same-iteration epilogues.
            for i in range(NIT + 3):
                if i < NIT:
                    e, g = eg(i)
                    if i in pre:
                        xg, t4 = pre[i]
                    else:
                        xg = xin.tile([64, cap], mmdt, tag="xg")
                        nc.sync.dma_start(out=xg[:], in_=xt[:, e, g, :])
                        t4 = tin.tile([64, 4, 128], mmdt, tag="t4")
                        nc.sync.dma_start(out=t4[:], in_=th[:, e, g])
                    qs = []
                    for ph in range(4):
                        q = qp.tile([128, cap], f32, tag="q")
                        nc.tensor.matmul(
                            q[:], t4[:, ph, :], xg[:], start=True, stop=True
                        )
                        qs.append(q)
                    st[i] = {"qs": qs}

                if i - 1 >= 0 and i - 1 < NIT:
                    j = i - 1
                    e, g = eg(j)
                    zpt = dpool.tile([8, cap], f32, tag="dzyo")
                    nc.tensor.matmul(
                        zpt[:], wE[:, e, g, :], st[j]["feat"][:],
                        start=True, stop=True,
                    )
                    st[j]["zp"] = zpt

                if i - 2 >= 0 and i - 2 < NIT:
                    j = i - 2
                    e, g = eg(j)
                    ypt = dpool.tile([128, cap], f32, tag="dzyo")
                    nc.tensor.matmul(
                        ypt[:], wD[:, e, g, :], st[j]["z"][:],
                        start=True, stop=True,
                    )
                    st[j]["ypp"] = ypt

                if i - 3 >= 0:
                    j = i - 3
                    e, g = eg(j)
                    opt = dpool.tile([64, cap], f32, tag="dzyo")
                    nc.tensor.matmul(
                        opt[:], tD[:, e, g, :], st[j]["y"][:],
                        start=True, stop=True,
                    )
                    o = ob.tile([64, cap], f32, tag="os")
                    nc.vector.tensor_scalar(
                        o[:], opt[:], bT[:, e, g : g + 1], 0.0, add, mx
                    )
                    nc.scalar.dma_start(out=out[e, g], in_=o[:])
                    del st[j]

                # epilogues (emitted after the PE block so engine streams
                # pick them up while PE moves on)
                if i < NIT:
                    qs = st[i]["qs"]
                    s0 = sb.tile([128, cap], f32, tag="s0")
                    nc.scalar.activation(s0[:], qs[0][:], relu)
                    m01 = sb.tile([128, cap], f32, tag="m01")
                    nc.vector.scalar_tensor_tensor(
                        m01[:], qs[1][:], 0.0, s0[:], op0=mx, op1=mx
                    )
                    m012 = sb.tile([128, cap], f32, tag="m012")
                    nc.vector.scalar_tensor_tensor(
                        m012[:], qs[2][:], 0.0, m01[:], op0=mx, op1=mx
                    )
                    feat = sb.tile([128, cap], mmdt, tag="feat")
                    nc.vector.scalar_tensor_tensor(
                        feat[:], qs[3][:], 0.0, m012[:], op0=mx, op1=mx
                    )
                    st[i]["feat"] = feat

                if i - 1 >= 0 and i - 1 < NIT:
                    j = i - 1
                    e, g = eg(j)
                    z = sb.tile([8, cap], mmdt, tag="zs")
                    nc.scalar.activation(
                        z[:], st[j]["zp"][:], relu, bias=bE[:, e, g : g + 1]
                    )
                    st[j]["z"] = z

                if i - 2 >= 0 and i - 2 < NIT:
                    j = i - 2
                    e, g = eg(j)
                    y = sb.tile([128, cap], mmdt, tag="ys")
                    nc.scalar.activation(
                        y[:], st[j]["ypp"][:], relu, bias=bD[:, e, g : g + 1]
                    )
                    st[j]["y"] = y

    nc.compile()
    return nc


# ---------------------------------------------------------------- numpy fallback
def _numpy_fallback(x, keys, t_enc, t_dec, edw, edb, ddw, ddb, dtb):
    n = x.shape[0]
    xf = x.reshape(n, G, 64)
    res = np.zeros((n, G, 64), np.float32)
    for e in range(E):
        idx = np.where(keys == e)[0]
        if len(idx) == 0:
            continue
        for g in range(G):
            xg = xf[idx, g].T
            q = [t_enc[e, g, ph].T @ xg for ph in range(4)]
            feat = np.maximum(
                np.maximum(np.maximum(q[0], q[1]), np.maximum(q[2], q[3])), 0.0
            )
            z = np.maximum(edw[e, g] @ feat + edb[e, g][:, None], 0.0)
            y = np.maximum(ddw[e, g] @ z + ddb[e, g][:, None], 0.0)
            o = np.maximum(t_dec[e, g].T @ y + dtb[e, g], 0.0)
            res[idx, g] = o.T
    return res.reshape(n, G, 8, 8)


# ---------------------------------------------------------------- entry point
def kernel(
    x,
    keys,
    conv_mask,
    enc_conv_w,
    enc_dense_w,
    enc_dense_b,
    dec_dense_w,
    dec_dense_b,
    dec_tconv_w,
    dec_tconv_b,
):
    x = np.asarray(x, np.float32)
    keys_np = np.asarray(keys).astype(np.int64)
    conv_mask = np.asarray(conv_mask, np.float32)
    enc_conv_w = np.asarray(enc_conv_w, np.float32)
    edw = np.asarray(enc_dense_w, np.float32)
    edb = np.asarray(enc_dense_b, np.float32)
    ddw = np.asarray(dec_dense_w, np.float32)
    ddb = np.asarray(dec_dense_b, np.float32)
    dtw = np.asarray(dec_tconv_w, np.float32)
    dtb = np.asarray(dec_tconv_b, np.float32)

    n = x.shape[0]
    t_enc = _build_t_enc(conv_mask, enc_conv_w)
    t_dec = _build_t_dec(dtw)

    counts = np.bincount(keys_np, minlength=E)
    cap = int(-(-max(counts.max(), 16) // 8) * 8)
    if cap > 512:
        return _numpy_fallback(x, keys_np, t_enc, t_dec, edw, edb, ddw, ddb, dtb)

    order = np.argsort(keys_np, kind="stable")
    starts = np.concatenate([[0], np.cumsum(counts)])
    xs = x.reshape(n, G, 64)[order]  # sorted by expert

    # bucketed input, conv-matmul layout [pix, e, g, slot]
    xh = np.zeros((E, G, 64, cap), np.float32)
    for e in range(E):
        ne = counts[e]
        if ne:
            xh[e, :, :, :ne] = xs[starts[e] : starts[e + 1]].transpose(1, 2, 0)

    # global weight hosts in exact SBUF layouts (partition dim first)
    mmnp = np.float16 if MM_DTYPE == "float16" else np.float32
    thg = np.ascontiguousarray(t_enc.transpose(3, 0, 1, 2, 4), mmnp)  # [64,E,G,4,128]
    wehg = np.ascontiguousarray(edw.transpose(3, 0, 1, 2), mmnp)  # [128,E,G,8]
    behg = np.ascontiguousarray(edb.transpose(2, 0, 1))  # [8,E,G]
    wdhg = np.ascontiguousarray(ddw.transpose(3, 0, 1, 2), mmnp)  # [8,E,G,128]
    bdhg = np.ascontiguousarray(ddb.transpose(2, 0, 1))  # [128,E,G]
    tdhg = np.ascontiguousarray(t_dec.transpose(2, 0, 1, 3), mmnp)  # [128,E,G,64]
    bthg = np.ascontiguousarray(
        np.broadcast_to(dtb[None, :, :], (64, E, G))
    )  # [64,E,G]
    xhg = np.ascontiguousarray(xh.transpose(2, 0, 1, 3), mmnp)  # [64,E,G,cap]

    cache_key = (cap, MM_DTYPE)
    if cache_key not in _PROG_CACHE:
        _PROG_CACHE[cache_key] = _build_program(cap)
    nc = _PROG_CACHE[cache_key]

    def core_slice(a, c):
        return np.ascontiguousarray(a[:, EPC * c : EPC * (c + 1)])

    in_maps = [
        {
            "xt": core_slice(xhg, c),
            "th": core_slice(thg, c),
            "weh": core_slice(wehg, c),
            "beh": core_slice(behg, c),
            "wdh": core_slice(wdhg, c),
            "bdh": core_slice(bdhg, c),
            "tdh": core_slice(tdhg, c),
            "bth": core_slice(bthg, c),
        }
        for c in range(N_CORES)
    ]

    global _LAST_IN_MAPS
    _LAST_IN_MAPS = in_maps

    from concourse.bass_utils import run_bass_kernel_spmd

    res = run_bass_kernel_spmd(nc, in_maps, core_ids=list(range(N_CORES)))

    og = np.concatenate([res.results[c]["out"] for c in range(N_CORES)], axis=0)
    result = np.empty((n, G, 64), np.float32)
    for e in range(E):
        ne = counts[e]
        if ne:
            result[order[starts[e] : starts[e + 1]]] = og[e, :, :, :ne].transpose(
                2, 0, 1
            )
    return result.reshape(n, G, 8, 8)

